# revision 10
# baseline (speedup 1.0000x reference)
"""4-bit column-block-quantized linear (ColBlockQuantizedLinear) on 8 TRN2 cores.

Math:  out[b,o] = scales[o] * (sum_i inp[b,i]*wq[o,i] - zeros[o]*rowsum[b])
with packed bytes q[j,o] (j = i//2): low nibble l = wq[o,2j], high nibble
h = wq[o,2j+1].  Identity: sum_j a_j*l_j + b_j*h_j = sum_j a_j*q_j + c_j*h_j
with c = b - 16a, q = 16h + l.

Device scheme (fp16 bit-trick): the fp16 bit pattern 0x5800|x encodes the
value 128 + x/8 EXACTLY for any 8-bit x.  So each weight stream is ONE
dual-op DVE tensor_scalar pass over the packed u16 data:
    Qlo = (q16 & 0x00FF) | 0x5800   -> 128 + q_lo/8      (pairs with 8a)
    Qhi = (q16 >>  8)    | 0x5800   -> 128 + q_hi/8      (pairs with 8a)
    Hlo = (q16 & 0x00F0) | 0x5800   -> 128 + 2*h_lo      (pairs with c/2)
    Hhi = (q16 >> 12)    | 0x5800   -> 128 + h_hi/8      (pairs with 8c)
The +128 offsets cancel exactly against rank-1 rows built from the SAME
fp16-rounded stationaries, folded with -zeros*rowsum into a K=9 fp16 hi/lo
correction matmul issued LAST (start=True sits on the first stream matmuls,
the corrections run warm and overlap the per-block scale/DMA tail).
Stationary activation factors are single fp16.  Scales are applied on-device
by per-psum-block DVE tensor_tensor multiplies into one output tile.

DMA: per-packet scheduling limits a single queue to ~100GB/s (one packet per
SBUF partition row), so weight chunks round-robin over the sync/scalar/gpsimd
queues, the first chunk and the stationaries are partition-split across all
three, and later chunks are WIDE (4 kt) so packets carry 5.5KB.  A few zero
matmuls on a memset tile warm the PE (HAM un-throttle needs ~3.4us of
sustained activity) while the first DMAs/DVE passes run.

Host byte layout: per core packed bytes [2048, 1376] are column-paired as
(m, 688+m) into uint16, rows regrouped so q_dram[r, kt*688+m] holds
contraction row kt*128+r -> contiguous per-partition DMA lines per chunk.

Sharding: column-parallel over out_features (1376 rows/core), inputs
replicated; per-core output [16,1376] gathered on host.
"""

import numpy as np

B = 16
I = 4096
O = 11008
NCORES = 8
OS = O // NCORES          # 1376 out-features per core
HOS = OS // 2             # 688 packed u16 columns
HALF = I // 2             # 2048 packed (contraction) rows
KT = HALF // 128          # 16 contraction tiles
CHUNKS = [1, 1, 2, 4, 4, 4]  # kt tiles per processing chunk
KC = 9                    # correction matmul contraction size
NDUMMY = 5                # PE warmup matmuls
# psum o-blocks (each within one 688-column half, <=512 cols per fp32 bank)
BLKS = [(0, 512), (512, 176), (688, 512), (1200, 176)]
PSPLIT = [0, 43, 86, 128]  # partition split for 3-queue DMAs

F16 = np.float16

_CACHE = {}


def _build_program():
    import concourse.bacc as bacc
    import concourse.mybir as mybir
    import concourse.tile as tile

    dt = mybir.dt
    op = mybir.AluOpType
    nc = bacc.Bacc("TRN2", target_bir_lowering=False)

    q = nc.dram_tensor("q", [128, KT * HOS], dt.uint16, kind="ExternalInput")
    stat = nc.dram_tensor("stat", [128, KT * 48], dt.float16, kind="ExternalInput")
    corr = nc.dram_tensor("corr", [KC, 16 + OS], dt.float16, kind="ExternalInput")
    sc = nc.dram_tensor("sc", [B, OS], dt.float32, kind="ExternalInput")
    out = nc.dram_tensor("out", [B, OS], dt.float32, kind="ExternalOutput")

    cstart = [0]
    for w in CHUNKS:
        cstart.append(cstart[-1] + w)

    with tile.TileContext(nc) as tc:
        with (
            tc.tile_pool(name="consts", bufs=1) as cpool,
            tc.tile_pool(name="qp", bufs=2) as qpool,
            tc.tile_pool(name="wp", bufs=2) as wpool,
            tc.tile_pool(name="op", bufs=1) as opool,
            tc.tile_pool(name="ps", bufs=1, space="PSUM") as pspool,
        ):
            QS = [nc.sync, nc.scalar, nc.gpsimd]  # DMA issue queues

            # PE warmup: zero matmuls while DMAs/DVE fill the pipeline
            dummy = cpool.tile([128, 512], dt.float16, name="dummy")
            ps_w = pspool.tile([16, 512], dt.float32, name="ps_w")
            nc.vector.memset(dummy, 0.0)
            for _ in range(NDUMMY):
                nc.tensor.matmul(
                    ps_w, dummy[:, 0:16], dummy, start=True, stop=True,
                    skip_group_check=True,
                )

            # first weight chunk: partition-split across all three queues
            qt0 = qpool.tile([128, CHUNKS[0] * HOS], dt.uint16, name="qt0", tag="qt1")
            for qi in range(3):
                p0, p1 = PSPLIT[qi], PSPLIT[qi + 1]
                QS[qi].dma_start(qt0[p0:p1, :], q[p0:p1, 0 : CHUNKS[0] * HOS])

            # stationaries: partition-split across all three queues
            stat_sb = cpool.tile([128, KT * 48], dt.float16, name="stat_sb")
            for qi in range(3):
                p0, p1 = PSPLIT[qi], PSPLIT[qi + 1]
                QS[qi].dma_start(stat_sb[p0:p1, :], stat[p0:p1, :])

            corr_sb = cpool.tile([KC, 16 + OS], dt.float16, name="corr_sb")
            sc_sb = cpool.tile([B, OS], dt.float32, name="sc_sb")
            nc.gpsimd.dma_start(corr_sb, corr[:, :])
            nc.gpsimd.dma_start(sc_sb, sc[:, :])
            corrL_sb = corr_sb[:, 0:16]
            corrR_sb = corr_sb[:, 16 : 16 + OS]

            psums = [
                pspool.tile([B, n], dt.float32, name=f"ps{i}")
                for i, (s, n) in enumerate(BLKS)
            ]

            for ci, cw in enumerate(CHUNKS):
                k0, w = cstart[ci], cw * HOS
                if ci == 0:
                    qt = qt0
                else:
                    qt = qpool.tile([128, w], dt.uint16, name=f"qt{ci}", tag=f"qt{cw}")
                    QS[(ci - 1) % 3].dma_start(qt, q[:, k0 * HOS : k0 * HOS + w])
                qlo = wpool.tile([128, w], dt.uint16, name=f"qlo{ci}", tag=f"qlo{cw}")
                qhi = wpool.tile([128, w], dt.uint16, name=f"qhi{ci}", tag=f"qhi{cw}")
                hlo = wpool.tile([128, w], dt.uint16, name=f"hlo{ci}", tag=f"hlo{cw}")
                hhi = wpool.tile([128, w], dt.uint16, name=f"hhi{ci}", tag=f"hhi{cw}")
                nc.vector.tensor_scalar(
                    qlo, qt, 0x00FF, 0x5800, op.bitwise_and, op.bitwise_or
                )
                nc.vector.tensor_scalar(
                    qhi, qt, 8, 0x5800, op.logical_shift_right, op.bitwise_or
                )
                nc.vector.tensor_scalar(
                    hlo, qt, 0x00F0, 0x5800, op.bitwise_and, op.bitwise_or
                )
                nc.vector.tensor_scalar(
                    hhi, qt, 12, 0x5800, op.logical_shift_right, op.bitwise_or
                )
                qlo16 = qlo.bitcast(dt.float16)
                qhi16 = qhi.bitcast(dt.float16)
                hlo16 = hlo.bitcast(dt.float16)
                hhi16 = hhi.bitcast(dt.float16)
                for h in range(cw):
                    kt = k0 + h
                    first = kt == 0
                    off = h * HOS
                    sq = stat_sb[:, kt * 48 : kt * 48 + 16]
                    shlo = stat_sb[:, kt * 48 + 16 : kt * 48 + 32]
                    shhi = stat_sb[:, kt * 48 + 32 : kt * 48 + 48]
                    for i, (s, n) in enumerate(BLKS):
                        if s < HOS:
                            a, b_ = off + s, off + s + n
                            nc.tensor.matmul(
                                psums[i], sq, qlo16[:, a:b_],
                                start=first, stop=False,
                            )
                            nc.tensor.matmul(
                                psums[i], shlo, hlo16[:, a:b_],
                                start=False, stop=False,
                            )
                        else:
                            a, b_ = off + s - HOS, off + s - HOS + n
                            nc.tensor.matmul(
                                psums[i], sq, qhi16[:, a:b_],
                                start=first, stop=False,
                            )
                            nc.tensor.matmul(
                                psums[i], shhi, hhi16[:, a:b_],
                                start=False, stop=False,
                            )

            # corrections last (PE is warm): -128*sum(coef) and -zeros*rowsum
            o = opool.tile([B, OS], dt.float32, name="o")
            for i, (s, n) in enumerate(BLKS):
                nc.tensor.matmul(
                    psums[i], corrL_sb, corrR_sb[:, s : s + n],
                    start=False, stop=True,
                )
                nc.vector.tensor_tensor(
                    o[:, s : s + n], psums[i], sc_sb[:, s : s + n], op.mult
                )
            nc.sync.dma_start(out[:, 0:HOS], o[:, 0:HOS])
            nc.scalar.dma_start(out[:, HOS:OS], o[:, HOS:OS])

    nc.finalize()
    return nc


def _get_program():
    if "nc" not in _CACHE:
        _CACHE["nc"] = _build_program()
    return _CACHE["nc"]


def _split_hi_lo(x64):
    hi = x64.astype(F16)
    lo = (x64 - hi.astype(np.float64)).astype(F16)
    return hi, lo


def _host_prep(inp, quant_weight, scales, zeros):
    """Per-core input maps: layout/precision prep only, no O(O*I) math."""
    inp64 = np.asarray(inp, dtype=np.float64)
    a = inp64[:, 0::2].T  # [HALF, B] even-i activations (pair with l / q)
    b = inp64[:, 1::2].T  # [HALF, B] odd-i activations (pair with h)
    c = b - 16.0 * a

    sq = (8.0 * a).astype(F16)      # [HALF, B]
    shlo = (c / 2.0).astype(F16)
    shhi = (8.0 * c).astype(F16)

    stat = np.zeros((128, KT * 48), dtype=F16)
    for kt in range(KT):
        rows = slice(kt * 128, (kt + 1) * 128)
        stat[:, kt * 48 : kt * 48 + 16] = sq[rows]
        stat[:, kt * 48 + 16 : kt * 48 + 32] = shlo[rows]
        stat[:, kt * 48 + 32 : kt * 48 + 48] = shhi[rows]

    # correction batch vectors from the ROUNDED stationaries (exact cancel)
    sum_sq = sq.astype(np.float64).sum(axis=0)      # [B]
    sum_shlo = shlo.astype(np.float64).sum(axis=0)
    sum_shhi = shhi.astype(np.float64).sum(axis=0)
    rowsum = inp64.sum(axis=1)                      # [B]
    sq_h, sq_l = _split_hi_lo(sum_sq)
    slo_h, slo_l = _split_hi_lo(sum_shlo)
    shi_h, shi_l = _split_hi_lo(sum_shhi)
    rs_h, rs_l = _split_hi_lo(rowsum)
    corrL = np.zeros((KC, 16), dtype=F16)
    corrL[0], corrL[1] = sq_h, sq_l
    corrL[2], corrL[3] = slo_h, slo_l
    corrL[4], corrL[5] = shi_h, shi_l
    corrL[6], corrL[7] = rs_h, rs_h
    corrL[8] = rs_l

    qw = np.asarray(quant_weight)
    scales = np.asarray(scales, dtype=np.float64).reshape(-1)
    zeros = np.asarray(zeros, dtype=np.float64).reshape(-1)

    in_maps = []
    for cidx in range(NCORES):
        rows = slice(cidx * OS, (cidx + 1) * OS)
        qc = qw[rows].astype(np.uint8).T  # [HALF, OS] natural columns
        # byte-pair columns (m, 688+m) -> uint16 elements
        qc2 = np.empty((HALF, OS), dtype=np.uint8)
        qc2[:, 0::2] = qc[:, :HOS]
        qc2[:, 1::2] = qc[:, HOS:]
        qu16 = np.ascontiguousarray(qc2).view(np.uint16)  # [HALF, HOS]
        # regroup rows: q_dram[r, kt*HOS + m] = qu16[kt*128 + r, m]
        q_dram = np.ascontiguousarray(
            qu16.reshape(KT, 128, HOS).transpose(1, 0, 2).reshape(128, KT * HOS)
        )
        z = zeros[rows]
        z_h, z_l = _split_hi_lo(z)
        corr_c = np.zeros((KC, 16 + OS), dtype=F16)
        corr_c[:, 0:16] = corrL
        corrR = corr_c[:, 16:]
        corrR[0] = -128.0
        corrR[1] = -128.0
        corrR[2, :HOS] = -128.0
        corrR[3, :HOS] = -128.0
        corrR[4, HOS:] = -128.0
        corrR[5, HOS:] = -128.0
        corrR[6] = -z_h
        corrR[7] = -z_l
        corrR[8] = -z_h
        sc_c = np.broadcast_to(scales[rows].astype(np.float32), (B, OS)).copy()
        in_maps.append(
            {
                "q": q_dram,
                "stat": stat,
                "corr": corr_c,
                "sc": sc_c,
            }
        )
    return in_maps


def kernel(inp, quant_weight, scales, zeros):
    from concourse.bass_utils import run_bass_kernel_spmd

    nc = _get_program()
    in_maps = _host_prep(inp, quant_weight, scales, zeros)
    res = run_bass_kernel_spmd(nc, in_maps, core_ids=list(range(NCORES)))
    out = np.concatenate(
        [res.results[c]["out"] for c in range(NCORES)], axis=1
    )
    return np.ascontiguousarray(out.astype(np.float32))


# revision 11
# speedup vs baseline: 1.2040x; 1.2040x over previous
"""4-bit column-block-quantized linear (ColBlockQuantizedLinear) on 8 TRN2 cores.

Math:  out[b,o] = scales[o] * (sum_i inp[b,i]*wq[o,i] - zeros[o]*rowsum[b])
with packed bytes q[j,o] (j = i//2): low nibble l = wq[o,2j], high nibble
h = wq[o,2j+1].  Identity: sum_j a_j*l_j + b_j*h_j = sum_j a_j*q_j + c_j*h_j
with c = b - 16a, q = 16h + l.

Device scheme (fp16 bit-trick): the fp16 bit pattern 0x5800|x encodes the
value 128 + x/8 EXACTLY for any 8-bit x.  So each weight stream is ONE
dual-op DVE tensor_scalar pass over the packed u16 data:
    Qlo = (q16 & 0x00FF) | 0x5800   -> 128 + q_lo/8      (pairs with 8a)
    Qhi = (q16 >>  8)    | 0x5800   -> 128 + q_hi/8      (pairs with 8a)
    Hlo = (q16 & 0x00F0) | 0x5800   -> 128 + 2*h_lo      (pairs with c/2)
    Hhi = (q16 >> 12)    | 0x5800   -> 128 + h_hi/8      (pairs with 8c)
The +128 offsets cancel exactly against rank-1 rows built from the SAME
fp16-rounded stationaries, folded with -zeros*rowsum into a K=9 fp16 hi/lo
correction matmul issued LAST (start=True sits on the first stream matmuls;
the corrections run warm and overlap the per-block scale/DMA tail).
Stationary activation factors are single fp16.  Scales are applied on-device
by per-psum-block DVE tensor_tensor multiplies into one output tile.

DMA economics: queues process ~one packet per SBUF partition row and
round-robin between in-flight descriptors, so (a) the first weight chunk is
FUSED with the stationaries into one 128-row DMA on the otherwise-empty sync
queue (payload is free, packets are not), (b) corr+scales are fused into one
16-row DMA, (c) bulk weight chunks are WIDE (3-4 kt -> 4-5.5KB packets,
bursts measured ~270GB/s) and spread over the scalar/gpsimd/sync queues so
no early-needed transfer queues behind a bulk one.  Eight zero matmuls on a
memset tile warm the PE (HAM un-throttle needs ~3.4us of sustained activity)
exactly while the first DMA + DVE passes run, so streams start at 2.4GHz.

Host byte layout: per core packed bytes [2048, 1376] are column-paired as
(m, 688+m) into uint16, rows regrouped so q chunk c's row r holds contraction
rows {kt*128+r} side by side -> contiguous per-partition DMA lines.

Sharding: column-parallel over out_features (1376 rows/core), inputs
replicated; per-core output [16,1376] gathered on host.
"""

import numpy as np

B = 16
I = 4096
O = 11008
NCORES = 8
OS = O // NCORES          # 1376 out-features per core
HOS = OS // 2             # 688 packed u16 columns
HALF = I // 2             # 2048 packed (contraction) rows
KT = HALF // 128          # 16 contraction tiles
CHUNKS = [1, 2, 2, 4, 4, 3]  # kt tiles per processing chunk
KC = 9                    # correction matmul contraction size
NDUMMY = 8                # PE warmup matmuls
SW = KT * 48              # stat columns (fp16)
# psum o-blocks (each within one 688-column half, <=512 cols per fp32 bank)
BLKS = [(0, 512), (512, 176), (688, 512), (1200, 176)]

F16 = np.float16

_CACHE = {}


def _build_program():
    import concourse.bacc as bacc
    import concourse.mybir as mybir
    import concourse.tile as tile

    dt = mybir.dt
    op = mybir.AluOpType
    nc = bacc.Bacc("TRN2", target_bir_lowering=False)

    # qs0 = [chunk0 | stat] fused; csc = [corr | scales] fused (u16 container)
    qs0 = nc.dram_tensor("qs0", [128, HOS + SW], dt.uint16, kind="ExternalInput")
    q = nc.dram_tensor("q", [128, (KT - 1) * HOS], dt.uint16, kind="ExternalInput")
    csc = nc.dram_tensor(
        "csc", [B, (16 + OS) + 2 * OS], dt.uint16, kind="ExternalInput"
    )
    out = nc.dram_tensor("out", [B, OS], dt.float32, kind="ExternalOutput")

    cstart = [0]
    for w in CHUNKS:
        cstart.append(cstart[-1] + w)
    # bulk chunk c (>=1) -> issue queue: c1 scalar, c2 gpsimd, c3 sync,
    # c4 scalar, c5 gpsimd
    QASSIGN = [None, "scalar", "gpsimd", "sync", "scalar", "gpsimd"]

    with tile.TileContext(nc) as tc:
        with (
            tc.tile_pool(name="consts", bufs=1) as cpool,
            tc.tile_pool(name="qp", bufs=2) as qpool,
            tc.tile_pool(name="wp", bufs=2) as wpool,
            tc.tile_pool(name="op", bufs=1) as opool,
            tc.tile_pool(name="ps", bufs=1, space="PSUM") as pspool,
        ):
            QS = {"sync": nc.sync, "scalar": nc.scalar, "gpsimd": nc.gpsimd}

            # PE warmup: zero matmuls while DMAs/DVE fill the pipeline
            dummy = cpool.tile([128, 512], dt.float16, name="dummy")
            ps_w = pspool.tile([16, 512], dt.float32, name="ps_w")
            nc.vector.memset(dummy, 0.0)
            for _ in range(NDUMMY):
                nc.tensor.matmul(
                    ps_w, dummy[:, 0:16], dummy, start=True, stop=True,
                    skip_group_check=True,
                )

            # fused first-chunk + stationaries (sync queue, nothing ahead)
            qs0_sb = cpool.tile([128, HOS + SW], dt.uint16, name="qs0_sb")
            nc.sync.dma_start(qs0_sb, qs0[:, :])
            stat16 = qs0_sb.bitcast(dt.float16)

            # fused corrections + scales (gpsimd queue, first)
            csc_sb = cpool.tile([B, 16 + OS + 2 * OS], dt.uint16, name="csc_sb")
            nc.gpsimd.dma_start(csc_sb, csc[:, :])
            corr16 = csc_sb.bitcast(dt.float16)
            corrL_sb = corr16[0:KC, 0:16]
            corrR_sb = corr16[0:KC, 16 : 16 + OS]
            sc_sb = csc_sb.bitcast(dt.float32)[:, (16 + OS) // 2 : (16 + OS) // 2 + OS]

            psums = [
                pspool.tile([B, n], dt.float32, name=f"ps{i}")
                for i, (s, n) in enumerate(BLKS)
            ]

            for ci, cw in enumerate(CHUNKS):
                k0, w = cstart[ci], cw * HOS
                if ci == 0:
                    qt = qs0_sb[:, 0:HOS]
                else:
                    qt = qpool.tile([128, w], dt.uint16, name=f"qt{ci}", tag=f"qt{cw}")
                    QS[QASSIGN[ci]].dma_start(
                        qt, q[:, (k0 - 1) * HOS : (k0 - 1) * HOS + w]
                    )
                qlo = wpool.tile([128, w], dt.uint16, name=f"qlo{ci}", tag=f"qlo{cw}")
                qhi = wpool.tile([128, w], dt.uint16, name=f"qhi{ci}", tag=f"qhi{cw}")
                hlo = wpool.tile([128, w], dt.uint16, name=f"hlo{ci}", tag=f"hlo{cw}")
                hhi = wpool.tile([128, w], dt.uint16, name=f"hhi{ci}", tag=f"hhi{cw}")
                nc.vector.tensor_scalar(
                    qlo, qt, 0x00FF, 0x5800, op.bitwise_and, op.bitwise_or
                )
                nc.vector.tensor_scalar(
                    qhi, qt, 8, 0x5800, op.logical_shift_right, op.bitwise_or
                )
                nc.vector.tensor_scalar(
                    hlo, qt, 0x00F0, 0x5800, op.bitwise_and, op.bitwise_or
                )
                nc.vector.tensor_scalar(
                    hhi, qt, 12, 0x5800, op.logical_shift_right, op.bitwise_or
                )
                qlo16 = qlo.bitcast(dt.float16)
                qhi16 = qhi.bitcast(dt.float16)
                hlo16 = hlo.bitcast(dt.float16)
                hhi16 = hhi.bitcast(dt.float16)
                for h in range(cw):
                    kt = k0 + h
                    first = kt == 0
                    off = h * HOS
                    sb = HOS + kt * 48  # stat base inside qs0 (f16 cols)
                    sq = stat16[:, sb : sb + 16]
                    shlo = stat16[:, sb + 16 : sb + 32]
                    shhi = stat16[:, sb + 32 : sb + 48]
                    for i, (s, n) in enumerate(BLKS):
                        if s < HOS:
                            a, b_ = off + s, off + s + n
                            nc.tensor.matmul(
                                psums[i], sq, qlo16[:, a:b_],
                                start=first, stop=False,
                            )
                            nc.tensor.matmul(
                                psums[i], shlo, hlo16[:, a:b_],
                                start=False, stop=False,
                            )
                        else:
                            a, b_ = off + s - HOS, off + s - HOS + n
                            nc.tensor.matmul(
                                psums[i], sq, qhi16[:, a:b_],
                                start=first, stop=False,
                            )
                            nc.tensor.matmul(
                                psums[i], shhi, hhi16[:, a:b_],
                                start=False, stop=False,
                            )

            # corrections last (PE is warm): -128*sum(coef) and -zeros*rowsum
            o = opool.tile([B, OS], dt.float32, name="o")
            for i, (s, n) in enumerate(BLKS):
                nc.tensor.matmul(
                    psums[i], corrL_sb, corrR_sb[:, s : s + n],
                    start=False, stop=True,
                )
                nc.vector.tensor_tensor(
                    o[:, s : s + n], psums[i], sc_sb[:, s : s + n], op.mult
                )
            nc.sync.dma_start(out[:, :], o)

    nc.finalize()
    return nc


def _get_program():
    if "nc" not in _CACHE:
        _CACHE["nc"] = _build_program()
    return _CACHE["nc"]


def _split_hi_lo(x64):
    hi = x64.astype(F16)
    lo = (x64 - hi.astype(np.float64)).astype(F16)
    return hi, lo


def _host_prep(inp, quant_weight, scales, zeros):
    """Per-core input maps: layout/precision prep only, no O(O*I) math."""
    inp64 = np.asarray(inp, dtype=np.float64)
    a = inp64[:, 0::2].T  # [HALF, B] even-i activations (pair with l / q)
    b = inp64[:, 1::2].T  # [HALF, B] odd-i activations (pair with h)
    c = b - 16.0 * a

    sq = (8.0 * a).astype(F16)      # [HALF, B]
    shlo = (c / 2.0).astype(F16)
    shhi = (8.0 * c).astype(F16)

    stat = np.zeros((128, SW), dtype=F16)
    for kt in range(KT):
        rows = slice(kt * 128, (kt + 1) * 128)
        stat[:, kt * 48 : kt * 48 + 16] = sq[rows]
        stat[:, kt * 48 + 16 : kt * 48 + 32] = shlo[rows]
        stat[:, kt * 48 + 32 : kt * 48 + 48] = shhi[rows]

    # correction batch vectors from the ROUNDED stationaries (exact cancel)
    sum_sq = sq.astype(np.float64).sum(axis=0)      # [B]
    sum_shlo = shlo.astype(np.float64).sum(axis=0)
    sum_shhi = shhi.astype(np.float64).sum(axis=0)
    rowsum = inp64.sum(axis=1)                      # [B]
    sq_h, sq_l = _split_hi_lo(sum_sq)
    slo_h, slo_l = _split_hi_lo(sum_shlo)
    shi_h, shi_l = _split_hi_lo(sum_shhi)
    rs_h, rs_l = _split_hi_lo(rowsum)
    corrL = np.zeros((KC, 16), dtype=F16)
    corrL[0], corrL[1] = sq_h, sq_l
    corrL[2], corrL[3] = slo_h, slo_l
    corrL[4], corrL[5] = shi_h, shi_l
    corrL[6], corrL[7] = rs_h, rs_h
    corrL[8] = rs_l

    qw = np.asarray(quant_weight)
    scales = np.asarray(scales, dtype=np.float64).reshape(-1)
    zeros = np.asarray(zeros, dtype=np.float64).reshape(-1)

    in_maps = []
    for cidx in range(NCORES):
        rows = slice(cidx * OS, (cidx + 1) * OS)
        qc = qw[rows].astype(np.uint8).T  # [HALF, OS] natural columns
        # byte-pair columns (m, 688+m) -> uint16 elements
        qc2 = np.empty((HALF, OS), dtype=np.uint8)
        qc2[:, 0::2] = qc[:, :HOS]
        qc2[:, 1::2] = qc[:, HOS:]
        qu16 = np.ascontiguousarray(qc2).view(np.uint16)  # [HALF, HOS]
        # regroup rows: chunked q[r, kt*HOS + m] = qu16[kt*128 + r, m]
        q_all = np.ascontiguousarray(
            qu16.reshape(KT, 128, HOS).transpose(1, 0, 2).reshape(128, KT * HOS)
        )
        qs0_c = np.concatenate(
            [q_all[:, 0:HOS], stat.view(np.uint16)], axis=1
        )
        q_c = np.ascontiguousarray(q_all[:, HOS:])

        z = zeros[rows]
        z_h, z_l = _split_hi_lo(z)
        corr_c = np.zeros((KC, 16 + OS), dtype=F16)
        corr_c[:, 0:16] = corrL
        corrR = corr_c[:, 16:]
        corrR[0] = -128.0
        corrR[1] = -128.0
        corrR[2, :HOS] = -128.0
        corrR[3, :HOS] = -128.0
        corrR[4, HOS:] = -128.0
        corrR[5, HOS:] = -128.0
        corrR[6] = -z_h
        corrR[7] = -z_l
        corrR[8] = -z_h
        sc_c = np.broadcast_to(scales[rows].astype(np.float32), (B, OS))
        csc_c = np.zeros((B, 16 + OS + 2 * OS), dtype=np.uint16)
        csc_c[0:KC, 0 : 16 + OS] = corr_c.view(np.uint16)
        csc_c[:, 16 + OS :] = np.ascontiguousarray(sc_c).view(np.uint16).reshape(
            B, 2 * OS
        )
        in_maps.append(
            {
                "qs0": qs0_c,
                "q": q_c,
                "csc": csc_c,
            }
        )
    return in_maps


def kernel(inp, quant_weight, scales, zeros):
    from concourse.bass_utils import run_bass_kernel_spmd

    nc = _get_program()
    in_maps = _host_prep(inp, quant_weight, scales, zeros)
    res = run_bass_kernel_spmd(nc, in_maps, core_ids=list(range(NCORES)))
    out = np.concatenate(
        [res.results[c]["out"] for c in range(NCORES)], axis=1
    )
    return np.ascontiguousarray(out.astype(np.float32))


# revision 14
# speedup vs baseline: 1.2454x; 1.0344x over previous
"""4-bit column-block-quantized linear (ColBlockQuantizedLinear) on 8 TRN2 cores.

Math:  out[b,o] = scales[o] * (sum_i inp[b,i]*wq[o,i] - zeros[o]*rowsum[b])
with packed bytes q[j,o] (j = i//2): low nibble l = wq[o,2j], high nibble
h = wq[o,2j+1].  Identity: sum_j a_j*l_j + b_j*h_j = sum_j a_j*q_j + c_j*h_j
with c = b - 16a, q = 16h + l.

Device scheme (fp16 bit-trick): the fp16 bit pattern 0x5800|x encodes the
value 128 + x/8 EXACTLY for any 8-bit x.  So each weight stream is ONE
dual-op DVE tensor_scalar pass over the packed u16 data:
    Qlo = (q16 & 0x00FF) | 0x5800   -> 128 + q_lo/8      (pairs with 8a)
    Qhi = (q16 >>  8)    | 0x5800   -> 128 + q_hi/8      (pairs with 8a)
    Hlo = (q16 & 0x00F0) | 0x5800   -> 128 + 2*h_lo      (pairs with c/2)
    Hhi = (q16 >> 12)    | 0x5800   -> 128 + h_hi/8      (pairs with 8c)
The +128 offsets cancel exactly against rank-1 rows built from the SAME
fp16-rounded stationaries, folded with -zeros*rowsum into a K=9 fp16 hi/lo
correction matmul issued LAST (start=True sits on the first stream matmuls;
the corrections run warm and overlap the per-block scale/DMA tail).
Stationary activation factors are single fp16.  Scales are applied on-device
by per-psum-block DVE tensor_tensor multiplies into one output tile.

DMA economics: queues process ~one packet per SBUF partition row and
round-robin between in-flight descriptors, so (a) the first weight chunk is
FUSED with the stationaries into one 128-row DMA on the otherwise-empty sync
queue (payload is free, packets are not), (b) corr+scales are fused into one
16-row DMA, (c) bulk weight chunks are WIDE (3-4 kt -> 4-5.5KB packets,
bursts measured ~270GB/s) and spread over the scalar/gpsimd/sync queues so
no early-needed transfer queues behind a bulk one.  Eight zero matmuls on a
memset tile warm the PE (HAM un-throttle needs ~3.4us of sustained activity)
exactly while the first DMA + DVE passes run, so streams start at 2.4GHz.

Host byte layout: per core packed bytes [2048, 1376] are column-paired as
(m, 688+m) into uint16, rows regrouped so q chunk c's row r holds contraction
rows {kt*128+r} side by side -> contiguous per-partition DMA lines.

Sharding: column-parallel over out_features (1376 rows/core), inputs
replicated; per-core output [16,1376] gathered on host.
"""

import numpy as np

B = 16
I = 4096
O = 11008
NCORES = 8
OS = O // NCORES          # 1376 out-features per core
HOS = OS // 2             # 688 packed u16 columns
HALF = I // 2             # 2048 packed (contraction) rows
KT = HALF // 128          # 16 contraction tiles
CHUNKS = [1, 2, 4, 4, 5]  # kt tiles per processing chunk
KC = 9                    # correction matmul contraction size
NDUMMY = 8                # PE warmup matmuls
SW = KT * 48              # stat columns (fp16)
# psum o-blocks (each within one 688-column half, <=512 cols per fp32 bank)
BLKS = [(0, 512), (512, 176), (688, 512), (1200, 176)]

F16 = np.float16

_CACHE = {}


def _build_program():
    import concourse.bacc as bacc
    import concourse.mybir as mybir
    import concourse.tile as tile

    dt = mybir.dt
    op = mybir.AluOpType
    nc = bacc.Bacc("TRN2", target_bir_lowering=False)

    # qs0 = [chunk0 | stat] fused; csc = [corr | scales] fused (u16 container)
    qs0 = nc.dram_tensor("qs0", [128, HOS + SW], dt.uint16, kind="ExternalInput")
    q = nc.dram_tensor("q", [128, (KT - 1) * HOS], dt.uint16, kind="ExternalInput")
    csc = nc.dram_tensor(
        "csc", [B, (16 + OS) + 2 * OS], dt.uint16, kind="ExternalInput"
    )
    out = nc.dram_tensor("out", [B, OS], dt.float32, kind="ExternalOutput")

    cstart = [0]
    for w in CHUNKS:
        cstart.append(cstart[-1] + w)
    # bulk chunk queues: each queue serves its chunks in need-order with at
    # most ~2 in flight, so round-robin between in-flight descriptors never
    # starves an early-deadline transfer behind a late bulk one
    QASSIGN = [None, "scalar", "sync", "scalar", "gpsimd"]

    with tile.TileContext(nc) as tc:
        with (
            tc.tile_pool(name="consts", bufs=1) as cpool,
            tc.tile_pool(name="qp", bufs=2) as qpool,
            tc.tile_pool(name="wp", bufs=2) as wpool,
            tc.tile_pool(name="op", bufs=1) as opool,
            tc.tile_pool(name="ps", bufs=1, space="PSUM") as pspool,
        ):
            QS = {"sync": nc.sync, "scalar": nc.scalar, "gpsimd": nc.gpsimd}

            # PE warmup: zero matmuls while DMAs/DVE fill the pipeline
            dummy = cpool.tile([128, 512], dt.float16, name="dummy")
            ps_w = pspool.tile([16, 512], dt.float32, name="ps_w")
            nc.vector.memset(dummy, 0.0)
            for _ in range(NDUMMY):
                nc.tensor.matmul(
                    ps_w, dummy[:, 0:16], dummy, start=True, stop=True,
                    skip_group_check=True,
                )

            # fused first-chunk + stationaries (sync queue, nothing ahead)
            qs0_sb = cpool.tile([128, HOS + SW], dt.uint16, name="qs0_sb")
            nc.sync.dma_start(qs0_sb, qs0[:, :])
            stat16 = qs0_sb.bitcast(dt.float16)

            # fused corrections + scales (gpsimd queue, first)
            csc_sb = cpool.tile([B, 16 + OS + 2 * OS], dt.uint16, name="csc_sb")
            nc.gpsimd.dma_start(csc_sb, csc[:, :])
            corr16 = csc_sb.bitcast(dt.float16)
            corrL_sb = corr16[0:KC, 0:16]
            corrR_sb = corr16[0:KC, 16 : 16 + OS]
            sc_sb = csc_sb.bitcast(dt.float32)[:, (16 + OS) // 2 : (16 + OS) // 2 + OS]

            psums = [
                pspool.tile([B, n], dt.float32, name=f"ps{i}")
                for i, (s, n) in enumerate(BLKS)
            ]

            for ci, cw in enumerate(CHUNKS):
                k0, w = cstart[ci], cw * HOS
                if ci == 0:
                    qt = qs0_sb[:, 0:HOS]
                else:
                    qt = qpool.tile([128, w], dt.uint16, name=f"qt{ci}", tag=f"qt{cw}")
                    QS[QASSIGN[ci]].dma_start(
                        qt, q[:, (k0 - 1) * HOS : (k0 - 1) * HOS + w]
                    )
                qlo = wpool.tile([128, w], dt.uint16, name=f"qlo{ci}", tag=f"qlo{cw}")
                qhi = wpool.tile([128, w], dt.uint16, name=f"qhi{ci}", tag=f"qhi{cw}")
                hlo = wpool.tile([128, w], dt.uint16, name=f"hlo{ci}", tag=f"hlo{cw}")
                hhi = wpool.tile([128, w], dt.uint16, name=f"hhi{ci}", tag=f"hhi{cw}")
                nc.vector.tensor_scalar(
                    qlo, qt, 0x00FF, 0x5800, op.bitwise_and, op.bitwise_or
                )
                nc.vector.tensor_scalar(
                    qhi, qt, 8, 0x5800, op.logical_shift_right, op.bitwise_or
                )
                nc.vector.tensor_scalar(
                    hlo, qt, 0x00F0, 0x5800, op.bitwise_and, op.bitwise_or
                )
                nc.vector.tensor_scalar(
                    hhi, qt, 12, 0x5800, op.logical_shift_right, op.bitwise_or
                )
                qlo16 = qlo.bitcast(dt.float16)
                qhi16 = qhi.bitcast(dt.float16)
                hlo16 = hlo.bitcast(dt.float16)
                hhi16 = hhi.bitcast(dt.float16)
                for h in range(cw):
                    kt = k0 + h
                    first = kt == 0
                    off = h * HOS
                    sb = HOS + kt * 48  # stat base inside qs0 (f16 cols)
                    sq = stat16[:, sb : sb + 16]
                    shlo = stat16[:, sb + 16 : sb + 32]
                    shhi = stat16[:, sb + 32 : sb + 48]
                    for i, (s, n) in enumerate(BLKS):
                        if s < HOS:
                            a, b_ = off + s, off + s + n
                            nc.tensor.matmul(
                                psums[i], sq, qlo16[:, a:b_],
                                start=first, stop=False,
                            )
                            nc.tensor.matmul(
                                psums[i], shlo, hlo16[:, a:b_],
                                start=False, stop=False,
                            )
                        else:
                            a, b_ = off + s - HOS, off + s - HOS + n
                            nc.tensor.matmul(
                                psums[i], sq, qhi16[:, a:b_],
                                start=first, stop=False,
                            )
                            nc.tensor.matmul(
                                psums[i], shhi, hhi16[:, a:b_],
                                start=False, stop=False,
                            )

            # corrections last (PE is warm): -128*sum(coef) and -zeros*rowsum
            o = opool.tile([B, OS], dt.float32, name="o")
            for i, (s, n) in enumerate(BLKS):
                nc.tensor.matmul(
                    psums[i], corrL_sb, corrR_sb[:, s : s + n],
                    start=False, stop=True,
                )
                nc.vector.tensor_tensor(
                    o[:, s : s + n], psums[i], sc_sb[:, s : s + n], op.mult
                )
                nc.gpsimd.dma_start(out[:, s : s + n], o[:, s : s + n])

    nc.finalize()
    return nc


def _get_program():
    if "nc" not in _CACHE:
        _CACHE["nc"] = _build_program()
    return _CACHE["nc"]


def _split_hi_lo(x64):
    hi = x64.astype(F16)
    lo = (x64 - hi.astype(np.float64)).astype(F16)
    return hi, lo


def _host_prep(inp, quant_weight, scales, zeros):
    """Per-core input maps: layout/precision prep only, no O(O*I) math."""
    inp64 = np.asarray(inp, dtype=np.float64)
    a = inp64[:, 0::2].T  # [HALF, B] even-i activations (pair with l / q)
    b = inp64[:, 1::2].T  # [HALF, B] odd-i activations (pair with h)
    c = b - 16.0 * a

    sq = (8.0 * a).astype(F16)      # [HALF, B]
    shlo = (c / 2.0).astype(F16)
    shhi = (8.0 * c).astype(F16)

    stat = np.zeros((128, SW), dtype=F16)
    for kt in range(KT):
        rows = slice(kt * 128, (kt + 1) * 128)
        stat[:, kt * 48 : kt * 48 + 16] = sq[rows]
        stat[:, kt * 48 + 16 : kt * 48 + 32] = shlo[rows]
        stat[:, kt * 48 + 32 : kt * 48 + 48] = shhi[rows]

    # correction batch vectors from the ROUNDED stationaries (exact cancel)
    sum_sq = sq.astype(np.float64).sum(axis=0)      # [B]
    sum_shlo = shlo.astype(np.float64).sum(axis=0)
    sum_shhi = shhi.astype(np.float64).sum(axis=0)
    rowsum = inp64.sum(axis=1)                      # [B]
    sq_h, sq_l = _split_hi_lo(sum_sq)
    slo_h, slo_l = _split_hi_lo(sum_shlo)
    shi_h, shi_l = _split_hi_lo(sum_shhi)
    rs_h, rs_l = _split_hi_lo(rowsum)
    corrL = np.zeros((KC, 16), dtype=F16)
    corrL[0], corrL[1] = sq_h, sq_l
    corrL[2], corrL[3] = slo_h, slo_l
    corrL[4], corrL[5] = shi_h, shi_l
    corrL[6], corrL[7] = rs_h, rs_h
    corrL[8] = rs_l

    qw = np.asarray(quant_weight)
    scales = np.asarray(scales, dtype=np.float64).reshape(-1)
    zeros = np.asarray(zeros, dtype=np.float64).reshape(-1)

    in_maps = []
    for cidx in range(NCORES):
        rows = slice(cidx * OS, (cidx + 1) * OS)
        qc = qw[rows].astype(np.uint8).T  # [HALF, OS] natural columns
        # byte-pair columns (m, 688+m) -> uint16 elements
        qc2 = np.empty((HALF, OS), dtype=np.uint8)
        qc2[:, 0::2] = qc[:, :HOS]
        qc2[:, 1::2] = qc[:, HOS:]
        qu16 = np.ascontiguousarray(qc2).view(np.uint16)  # [HALF, HOS]
        # regroup rows: chunked q[r, kt*HOS + m] = qu16[kt*128 + r, m]
        q_all = np.ascontiguousarray(
            qu16.reshape(KT, 128, HOS).transpose(1, 0, 2).reshape(128, KT * HOS)
        )
        qs0_c = np.concatenate(
            [q_all[:, 0:HOS], stat.view(np.uint16)], axis=1
        )
        q_c = np.ascontiguousarray(q_all[:, HOS:])

        z = zeros[rows]
        z_h, z_l = _split_hi_lo(z)
        corr_c = np.zeros((KC, 16 + OS), dtype=F16)
        corr_c[:, 0:16] = corrL
        corrR = corr_c[:, 16:]
        corrR[0] = -128.0
        corrR[1] = -128.0
        corrR[2, :HOS] = -128.0
        corrR[3, :HOS] = -128.0
        corrR[4, HOS:] = -128.0
        corrR[5, HOS:] = -128.0
        corrR[6] = -z_h
        corrR[7] = -z_l
        corrR[8] = -z_h
        sc_c = np.broadcast_to(scales[rows].astype(np.float32), (B, OS))
        csc_c = np.zeros((B, 16 + OS + 2 * OS), dtype=np.uint16)
        csc_c[0:KC, 0 : 16 + OS] = corr_c.view(np.uint16)
        csc_c[:, 16 + OS :] = np.ascontiguousarray(sc_c).view(np.uint16).reshape(
            B, 2 * OS
        )
        in_maps.append(
            {
                "qs0": qs0_c,
                "q": q_c,
                "csc": csc_c,
            }
        )
    return in_maps


def kernel(inp, quant_weight, scales, zeros):
    from concourse.bass_utils import run_bass_kernel_spmd

    nc = _get_program()
    in_maps = _host_prep(inp, quant_weight, scales, zeros)
    res = run_bass_kernel_spmd(nc, in_maps, core_ids=list(range(NCORES)))
    out = np.concatenate(
        [res.results[c]["out"] for c in range(NCORES)], axis=1
    )
    return np.ascontiguousarray(out.astype(np.float32))
